# revision 6
# baseline (speedup 1.0000x reference)
"""Chamfer loss kernel for Trainium2 (8 NeuronCores, SPMD).

Problem: B=4, N=M=8192, D=64 (fp32 in / fp32 scalar out).
  dist[b,n,m] = ||f[b,n] - f_[b,m]||^2
  out = mean_b( mean_n min_m dist + mean_m min_n dist )

Sharding: core c handles batch c//2, row-half c%2 (4096 rows x 8192 cols
of the distance matrix). Each core computes complete row-mins for its
4096 rows and partial col-mins (over its rows) for all 8192 cols; host
combines partials (min over the 2 cores per batch + means).

Device dataflow per core:
  - matmul (fp16, K=66): lhsT = [-2*f^T ; p ; 1], rhs = [f_^T ; 1 ; q-SHIFT]
    so PSUM tile = dist - SHIFT directly (rank-2 norm update rides the
    contraction).
  - A hand-authored custom DVE op (CHAMFER_MINMIN) does BOTH min passes in
    one read: out[:, :2048] = min(feed, C) updates the col accumulator,
    while a row-min accumulator rides the datapath and two drain FSM
    states append it as 2 extra output columns (C is [128, 2050]).
    On fp16 SBUF feeds it runs in the 2x_1p perf mode (~1.24us per tile,
    hand-written 2x uop program); with in0 = PSUM fp32 it falls back to
    the 1x program (~2.3us) and needs no ScalarE copy at all.
  - Tiles are split between the two paths to balance ScalarE (PSUM->SBUF
    fp16 drain, 1.85us/tile) against the DVE: ~1/5 of tiles go PSUM-direct.
  - Row-min pads are evicted per tile by tiny [128, 2] DMAs before the
    next op on the same group overwrites them.
"""

import os

import numpy as np

import concourse.bass as bass
import concourse.dve_ops as DO
import concourse.mybir as mybir
import concourse.tile as tile
from concourse import bacc
from concourse.bass import ts
from concourse.bass_utils import run_bass_kernel_spmd
from concourse.dve_spec import C0, AluOp as SpecAluOp, Spec, Src0, Src1, minn
from concourse.dve_uop import (
    AluInp,
    AluOp,
    DelayInp,
    DveOpSpec,
    InpSel,
    OutPath,
    OutSel,
    Trigger,
    UopConfig,
)

# --------------------------------------------------------------------------
# Custom DVE op: fused col-min tensor_tensor + row-min reduction
# --------------------------------------------------------------------------

OP_NAME = "CHAMFER_MINMIN"
DRAIN_A = 8  # drain cycles so the held row-min ripples blk3->blk7 (>=4)


def _reference(in0, in1, s0, s1, imm2):
    in0 = np.asarray(in0, np.float32)
    body = np.minimum(in0, np.asarray(in1, np.float32))
    rm = in0.reshape(in0.shape[0], -1).min(axis=-1, keepdims=True)
    if isinstance(s0, np.ndarray):
        rm = np.minimum(np.asarray(s0, np.float32).reshape(-1, 1), rm)
    else:
        rm = np.minimum(float(s0), rm)
    return np.concatenate([body, rm, rm], axis=1)


_SPEC = Spec(
    body=minn(Src0, Src1),
    accum=SpecAluOp.MIN,
    accum_init=C0,
    reference=_reference,
)


def _build_1x():
    def common(u: UopConfig):
        u.enable_input(InpSel.SRC_0, 1)
        u.enable_input(InpSel.SRC_1, 2)
        u.enable_input(InpSel.CONST_0, 3)
        b = u.datapath_config
        b[0].enable_alu(AluOp.MIN, AluInp.PREV_DELAY_0, AluInp.PREV_DELAY_1)
        b[0].pass_through_delay(0, 1, 2)
        b[1].enable_delay_from_src(DelayInp.PREV_ALU_OUT, 0)
        b[1].pass_through_delay(1, 2)
        for i in range(2, 8):
            b[i].pass_through_alu()
            b[i].pass_through_delay(0, 1, 2)
        return u

    seed = common(UopConfig())
    seed.datapath_config[1].enable_alu(
        AluOp.BYPASS, AluInp.PREV_DELAY_2, AluInp.PREV_DELAY_2
    )
    seed.repeat_count = 1
    seed.trigger = (Trigger.COUNT, Trigger.NONE, Trigger.NONE)
    seed.next_uop = (1, 0, 0)

    steady = common(UopConfig())
    steady.datapath_config[1].enable_alu(
        AluOp.MIN, AluInp.CURR_ALU_OUT, AluInp.PREV_DELAY_0
    )
    steady.trigger = (Trigger.SRC_TENSOR_DONE, Trigger.NONE, Trigger.NONE)
    steady.next_uop = (2, 0, 0)
    steady.require_inp0 = 1
    steady.require_inp1 = 1
    steady.enable_output(OutSel.DELAY_0, OutPath.WR0_LO)

    def drain(u: UopConfig):
        b = u.datapath_config
        b[1].enable_alu(AluOp.BYPASS, AluInp.CURR_ALU_OUT, AluInp.CURR_ALU_OUT)
        for i in range(2, 8):
            b[i].pass_through_alu()
        u.trigger = (Trigger.COUNT, Trigger.NONE, Trigger.NONE)
        return u

    drain_a = drain(UopConfig())
    drain_a.repeat_count = DRAIN_A
    drain_a.next_uop = (3, 0, 0)

    drain_b = drain(UopConfig())
    drain_b.repeat_count = 2
    drain_b.next_uop = (0, 0, 0)
    drain_b.enable_output(OutSel.ALU_OUT, OutPath.WR0_LO)

    return [seed, steady, drain_a, drain_b]


def _build_2x():
    def common(u: UopConfig):
        u.enable_input(InpSel.SRC_0, 0)
        u.enable_input(InpSel.SRC_1, 1)
        u.enable_input(InpSel.SRC_0_HI, 2)
        u.enable_input(InpSel.SRC_1_HI, 3)
        u.enable_input(InpSel.CONST_0, 4)
        b = u.datapath_config
        b[0].enable_alu(AluOp.MIN, AluInp.PREV_ALU_OUT, AluInp.PREV_DELAY_0)
        b[0].pass_through_delay(1, 2, 3)
        b[0].enable_delay_from_src(DelayInp.PREV_ALU_OUT, 4)
        b[1].enable_alu(AluOp.MIN, AluInp.PREV_DELAY_1, AluInp.PREV_DELAY_2)
        b[1].enable_delay_from_src(DelayInp.PREV_ALU_OUT, 0)
        b[1].pass_through_delay(1, 3, 4)
        b[2].enable_alu(AluOp.MIN, AluInp.PREV_DELAY_4, AluInp.PREV_DELAY_1)
        b[2].enable_delay_from_src(DelayInp.PREV_ALU_OUT, 2)
        b[2].pass_through_delay(0, 3)
        b[3].pass_through_delay(0, 2, 3)
        for i in range(4, 8):
            b[i].pass_through_alu()
            b[i].pass_through_delay(0, 2, 3)
        return u

    seed = common(UopConfig())
    seed.datapath_config[3].enable_alu(
        AluOp.BYPASS, AluInp.PREV_DELAY_3, AluInp.PREV_DELAY_3
    )
    seed.repeat_count = 1
    seed.trigger = (Trigger.COUNT, Trigger.NONE, Trigger.NONE)
    seed.next_uop = (1, 0, 0)

    steady = common(UopConfig())
    steady.datapath_config[3].enable_alu(
        AluOp.MIN, AluInp.CURR_ALU_OUT, AluInp.PREV_ALU_OUT
    )
    steady.trigger = (Trigger.SRC_TENSOR_DONE, Trigger.NONE, Trigger.NONE)
    steady.next_uop = (2, 0, 0)
    steady.require_inp0 = 1
    steady.require_inp1 = 1
    steady.enable_output(OutSel.DELAY_0, OutPath.WR0_LO)
    steady.enable_output(OutSel.DELAY_2, OutPath.WR0_HI)

    def drain(u: UopConfig):
        b = u.datapath_config
        b[3].enable_alu(AluOp.BYPASS, AluInp.CURR_ALU_OUT, AluInp.CURR_ALU_OUT)
        for i in range(4, 8):
            b[i].pass_through_alu()
        u.trigger = (Trigger.COUNT, Trigger.NONE, Trigger.NONE)
        return u

    drain_a = drain(UopConfig())
    drain_a.repeat_count = DRAIN_A
    drain_a.next_uop = (3, 0, 0)

    drain_b = drain(UopConfig())
    drain_b.repeat_count = 1
    drain_b.next_uop = (0, 0, 0)
    drain_b.enable_output(OutSel.ALU_OUT, OutPath.WR0_LO)
    drain_b.enable_output(OutSel.ALU_OUT, OutPath.WR0_HI)

    return [seed, steady, drain_a, drain_b]


class _FusedOp:
    name = OP_NAME
    spec = _SPEC
    subdim = False

    def __init__(self):
        self._cache = {}

    def compile(self, ver):
        if ver in self._cache:
            return self._cache[ver]
        assert ver == "v3", f"only TRN2/v3 supported, got {ver}"
        s = DveOpSpec(
            name=self.name,
            opcode=DO.get_dve_sub_opcode(self.name),
            uops=_build_1x(),
            uops_2x=_build_2x(),
            rd1_en=True,
            perf_max=1,
        )
        s.validate(ver)
        self._cache[ver] = s
        return s


def _register():
    if OP_NAME in DO._SUB_OPCODE_FOR_NAME:
        return next(op for op in DO.OPS if op.name == OP_NAME)
    op = _FusedOp()
    DO.OPS.append(op)
    DO._SUB_OPCODE_FOR_NAME[OP_NAME] = DO._CUSTOM_DVE_ROW_BASE + len(DO.OPS) - 1
    DO.CUSTOM_DVE_SPECS[OP_NAME] = _SPEC
    return op


FUSED_OP = _register()


def emit_fused(nc, out, in0, in1, s0):
    inst = nc.vector._custom_dve(FUSED_OP, out=out, in0=in0, in1=in1, s0=s0, s1=0.0)
    inst.ins.perf_max = 1  # BassInstruction wraps the rust instr as .ins
    return inst


# --------------------------------------------------------------------------
# Chamfer kernel
# --------------------------------------------------------------------------

B, N, M, D = 4, 8192, 8192, 64
N_CORES = 8
ROWS = N // 2          # rows per core (half a batch)
SHIFT = 48.0
BIGVAL = 60000.0       # row-min accumulator seed (fp16-safe "+inf")

P = 128                # n-tile height (PSUM partitions)
MB = 512               # m-block width (one PSUM bank of fp32)
GROUP = 4              # m-blocks per PSUM group tile ([128, 2048] = 4 banks)
PAD = 2                # row-min pad columns appended to each C group

# every k-th eligible tile (i>0) goes PSUM-direct on the DVE (no ACT copy)
K_PSUM = int(os.environ.get("CHAMFER_K_PSUM", "27"))

LAST_RESULTS = None    # test.py reads exec_time_ns / profile from here


def _build_program(rows=ROWS, cols=M):
    n_tiles = rows // P
    m_groups = cols // (MB * GROUP)
    GW = MB * GROUP        # feed-group width (2048)
    K = D + 2

    f16 = mybir.dt.float16
    f32 = mybir.dt.float32

    # choose PSUM-direct tiles: spread K_PSUM of the i>0 tiles evenly
    # (g-major linear index: lin = g * n_tiles + i; skip each group's i==0)
    n_lin = n_tiles * m_groups
    eligible = [t for t in range(n_lin) if t % n_tiles != 0]
    psum_path = set()
    if K_PSUM > 0:
        stride = len(eligible) / K_PSUM
        psum_path = {eligible[min(len(eligible) - 1, int(j * stride))]
                     for j in range(K_PSUM)}

    nc = bacc.Bacc()
    lhs_d = nc.dram_tensor("lhs", [K, rows], f16, kind="ExternalInput")
    rhs_d = nc.dram_tensor("rhs", [K, cols], f16, kind="ExternalInput")
    row_d = nc.dram_tensor("rowacc", [n_tiles, m_groups, P, PAD], f16,
                           kind="ExternalOutput")
    col_d = nc.dram_tensor("colmins", [P, cols], f16, kind="ExternalOutput")

    with tile.TileContext(nc) as tc:
        with (
            tc.tile_pool(name="const", bufs=1) as const_pool,
            tc.tile_pool(name="feed", bufs=8) as feed_pool,
            tc.tile_pool(name="psum", bufs=2, space="PSUM") as psum_pool,
        ):
            lhs_sb = const_pool.tile([K, rows], f16)
            rhs_sb = const_pool.tile([K, cols], f16)
            # chunked loads, ordered so the first matmul (lhs cols 0:128 +
            # rhs cols 0:512) gates on the first two DMAs, not the whole train
            nc.sync.dma_start(lhs_sb[:, 0:P], lhs_d[:, 0:P])
            for c in range(0, GW, MB):
                nc.sync.dma_start(rhs_sb[:, c:c + MB], rhs_d[:, c:c + MB])
            nc.sync.dma_start(lhs_sb[:, P:GW], lhs_d[:, P:GW])
            lhs_chunks = [(c, min(c + GW, rows)) for c in range(GW, rows, GW)]
            rhs_chunks = [(c, min(c + GW, cols)) for c in range(GW, cols, GW)]
            li = ri = 0
            while ri < len(rhs_chunks) or li < len(lhs_chunks):
                if ri < len(rhs_chunks):
                    c, e = rhs_chunks[ri]; ri += 1
                    nc.sync.dma_start(rhs_sb[:, c:e], rhs_d[:, c:e])
                if li < len(lhs_chunks):
                    c, e = lhs_chunks[li]; li += 1
                    nc.sync.dma_start(lhs_sb[:, c:e], lhs_d[:, c:e])

            # col-min accumulators: ping-pong pair per m-group so the pad
            # eviction DMA of tile i never WAR-blocks tile i+1's op
            Cs = [
                [
                    const_pool.tile([P, GW + PAD], f16, name=f"C{g}_{s}")
                    for s in range(2)
                ]
                for g in range(m_groups)
            ]

            # g-major loop: group g's col-min accumulator finalizes after its
            # 32 n-tiles, so its [128, 2048] store overlaps group g+1's
            # compute — only the last group's store is tail-exposed
            for g in range(m_groups):
                for i in range(n_tiles):
                    lin = g * n_tiles + i
                    lhs_i = lhs_sb[:, ts(i, P)]
                    Cg_out = Cs[g][i % 2]
                    Cg_in = Cs[g][(i + 1) % 2]
                    ps = psum_pool.tile([P, GW], f32)
                    for jj in range(GROUP):
                        j = g * GROUP + jj
                        nc.tensor.matmul(
                            ps[:, ts(jj, MB)],
                            lhs_i,
                            rhs_sb[:, ts(j, MB)],
                            start=True,
                            stop=True,
                        )
                    if lin in psum_path:
                        # DVE reads PSUM directly (1x program): drain + both
                        # min passes in one op, ScalarE untouched
                        emit_fused(nc, out=Cg_out[:], in0=ps[:],
                                   in1=Cg_in[:, 0:GW], s0=BIGVAL)
                    else:
                        sb = feed_pool.tile([P, GW], f16)
                        nc.scalar.copy(sb[:], ps[:])
                        if i == 0:
                            # C is uninitialized: min(feed, feed) = feed
                            emit_fused(nc, out=Cg_out[:], in0=sb[:],
                                       in1=sb[:], s0=BIGVAL)
                        else:
                            emit_fused(nc, out=Cg_out[:], in0=sb[:],
                                       in1=Cg_in[:, 0:GW], s0=BIGVAL)
                    # evict this tile's row-min pad (ping-pong: next op on
                    # this group writes the other buffer, so no WAR stall)
                    nc.sync.dma_start(row_d[i, g], Cg_out[:, GW:GW + PAD])
                last = (n_tiles - 1) % 2
                nc.sync.dma_start(col_d[:, ts(g, GW)], Cs[g][last][:, 0:GW])

    nc.finalize()
    return nc


_PROGRAM_CACHE = {}


def _get_program(rows=ROWS, cols=M):
    key = (rows, cols, K_PSUM)
    if key not in _PROGRAM_CACHE:
        _PROGRAM_CACHE[key] = _build_program(rows, cols)
    return _PROGRAM_CACHE[key]


def _prep_core_inputs(f, f_, core):
    """Host-side shard + layout: build augmented lhs/rhs for one core."""
    b, h = divmod(core, 2)
    fh = f[b, h * ROWS : (h + 1) * ROWS]          # [ROWS, D]
    g = f_[b]                                     # [M, D]
    p = np.einsum("nd,nd->n", fh, fh, dtype=np.float32)
    q = np.einsum("md,md->m", g, g, dtype=np.float32)

    K = D + 2
    lhs = np.empty((K, ROWS), np.float16)
    lhs[:D] = (-2.0 * fh.T).astype(np.float16)
    lhs[D] = p.astype(np.float16)
    lhs[D + 1] = 1.0

    rhs = np.empty((K, M), np.float16)
    rhs[:D] = g.T.astype(np.float16)
    rhs[D] = 1.0
    rhs[D + 1] = (q - SHIFT).astype(np.float16)
    return {"lhs": lhs, "rhs": rhs}


def kernel(f, f_):
    global LAST_RESULTS
    f = np.asarray(f, dtype=np.float32)
    f_ = np.asarray(f_, dtype=np.float32)

    in_maps = [_prep_core_inputs(f, f_, c) for c in range(N_CORES)]
    nc = _get_program()
    res = run_bass_kernel_spmd(
        nc,
        in_maps,
        list(range(N_CORES)),
        trace=bool(int(os.environ.get("CHAMFER_TRACE", "0"))),
    )
    LAST_RESULTS = res

    total = 0.0
    for b in range(B):
        r0 = res.results[2 * b]
        r1 = res.results[2 * b + 1]
        # rowacc[i, g, p, 0] = row-min over group g for row i*128 + p
        rm = np.concatenate(
            [
                r0["rowacc"][:, :, :, 0].astype(np.float32).min(axis=1).reshape(-1),
                r1["rowacc"][:, :, :, 0].astype(np.float32).min(axis=1).reshape(-1),
            ]
        ) + SHIFT
        cm = (
            np.minimum(
                r0["colmins"].astype(np.float32).min(axis=0),
                r1["colmins"].astype(np.float32).min(axis=0),
            )
            + SHIFT
        )
        total += rm.mean() + cm.mean()
    return np.asarray(total / B, dtype=np.float32)


# revision 10
# speedup vs baseline: 2.3089x; 2.3089x over previous
"""Chamfer loss kernel for Trainium2 (8 NeuronCores, SPMD).

Problem: B=4, N=M=8192, D=64 (fp32 in / fp32 scalar out).
  dist[b,n,m] = ||f[b,n] - f_[b,m]||^2
  out = mean_b( mean_n min_m dist + mean_m min_n dist )

Sharding: core c handles batch c//2, row-half c%2 (4096 rows x 8192 cols
of the distance matrix). Each core computes complete row-mins for its
4096 rows and partial col-mins (over its rows) for all 8192 cols; host
combines partials (min over the 2 cores per batch + means).

Device dataflow per core:
  - matmul (fp16, K=66): lhsT = [-2*f^T ; p ; 1], rhs = [f_^T ; 1 ; q-SHIFT]
    so PSUM tile = dist - SHIFT directly (rank-2 norm update rides the
    contraction).
  - A hand-authored custom DVE op (CHAMFER_MINMIN) does BOTH min passes in
    one read: out[:, :2048] = min(feed, C) updates the col accumulator,
    while a row-min accumulator rides the datapath and two drain FSM
    states append it as 2 extra output columns (C is [128, 2050]).
    On fp16 SBUF feeds it runs in the 2x_1p perf mode (~1.24us per tile,
    hand-written 2x uop program); with in0 = PSUM fp32 it falls back to
    the 1x program (~2.3us) and needs no ScalarE copy at all.
  - Tiles are split between the two paths to balance ScalarE (PSUM->SBUF
    fp16 drain, 1.85us/tile) against the DVE: ~1/5 of tiles go PSUM-direct.
  - Row-min pads are evicted per tile by tiny [128, 2] DMAs before the
    next op on the same group overwrites them.
"""

import os

import numpy as np

import concourse.bass as bass
import concourse.dve_ops as DO
import concourse.mybir as mybir
import concourse.tile as tile
from concourse import bacc
from concourse.bass import ts
from concourse.bass_utils import run_bass_kernel_spmd
from concourse.dve_spec import C0, AluOp as SpecAluOp, Spec, Src0, Src1, minn
from concourse.dve_uop import (
    AluInp,
    AluOp,
    DelayInp,
    DveOpSpec,
    InpSel,
    OutPath,
    OutSel,
    Trigger,
    UopConfig,
)

# --------------------------------------------------------------------------
# Custom DVE op: fused col-min tensor_tensor + row-min reduction
# --------------------------------------------------------------------------

OP_NAME = "CHAMFER_MINMIN"
DRAIN_A = 8  # drain cycles so the held row-min ripples blk3->blk7 (>=4)


def _reference(in0, in1, s0, s1, imm2):
    in0 = np.asarray(in0, np.float32)
    body = np.minimum(in0, np.asarray(in1, np.float32))
    rm = in0.reshape(in0.shape[0], -1).min(axis=-1, keepdims=True)
    if isinstance(s0, np.ndarray):
        rm = np.minimum(np.asarray(s0, np.float32).reshape(-1, 1), rm)
    else:
        rm = np.minimum(float(s0), rm)
    return np.concatenate([body, rm, rm], axis=1)


_SPEC = Spec(
    body=minn(Src0, Src1),
    accum=SpecAluOp.MIN,
    accum_init=C0,
    reference=_reference,
)


def _build_1x():
    def common(u: UopConfig):
        u.enable_input(InpSel.SRC_0, 1)
        u.enable_input(InpSel.SRC_1, 2)
        u.enable_input(InpSel.CONST_0, 3)
        b = u.datapath_config
        b[0].enable_alu(AluOp.MIN, AluInp.PREV_DELAY_0, AluInp.PREV_DELAY_1)
        b[0].pass_through_delay(0, 1, 2)
        b[1].enable_delay_from_src(DelayInp.PREV_ALU_OUT, 0)
        b[1].pass_through_delay(1, 2)
        for i in range(2, 8):
            b[i].pass_through_alu()
            b[i].pass_through_delay(0, 1, 2)
        return u

    seed = common(UopConfig())
    seed.datapath_config[1].enable_alu(
        AluOp.BYPASS, AluInp.PREV_DELAY_2, AluInp.PREV_DELAY_2
    )
    seed.repeat_count = 1
    seed.trigger = (Trigger.COUNT, Trigger.NONE, Trigger.NONE)
    seed.next_uop = (1, 0, 0)

    steady = common(UopConfig())
    steady.datapath_config[1].enable_alu(
        AluOp.MIN, AluInp.CURR_ALU_OUT, AluInp.PREV_DELAY_0
    )
    steady.trigger = (Trigger.SRC_TENSOR_DONE, Trigger.NONE, Trigger.NONE)
    steady.next_uop = (2, 0, 0)
    steady.require_inp0 = 1
    steady.require_inp1 = 1
    steady.enable_output(OutSel.DELAY_0, OutPath.WR0_LO)

    def drain(u: UopConfig):
        b = u.datapath_config
        b[1].enable_alu(AluOp.BYPASS, AluInp.CURR_ALU_OUT, AluInp.CURR_ALU_OUT)
        for i in range(2, 8):
            b[i].pass_through_alu()
        u.trigger = (Trigger.COUNT, Trigger.NONE, Trigger.NONE)
        return u

    drain_a = drain(UopConfig())
    drain_a.repeat_count = DRAIN_A
    drain_a.next_uop = (3, 0, 0)

    drain_b = drain(UopConfig())
    drain_b.repeat_count = 2
    drain_b.next_uop = (0, 0, 0)
    drain_b.enable_output(OutSel.ALU_OUT, OutPath.WR0_LO)

    return [seed, steady, drain_a, drain_b]


def _build_2x():
    def common(u: UopConfig):
        u.enable_input(InpSel.SRC_0, 0)
        u.enable_input(InpSel.SRC_1, 1)
        u.enable_input(InpSel.SRC_0_HI, 2)
        u.enable_input(InpSel.SRC_1_HI, 3)
        u.enable_input(InpSel.CONST_0, 4)
        b = u.datapath_config
        b[0].enable_alu(AluOp.MIN, AluInp.PREV_ALU_OUT, AluInp.PREV_DELAY_0)
        b[0].pass_through_delay(1, 2, 3)
        b[0].enable_delay_from_src(DelayInp.PREV_ALU_OUT, 4)
        b[1].enable_alu(AluOp.MIN, AluInp.PREV_DELAY_1, AluInp.PREV_DELAY_2)
        b[1].enable_delay_from_src(DelayInp.PREV_ALU_OUT, 0)
        b[1].pass_through_delay(1, 3, 4)
        b[2].enable_alu(AluOp.MIN, AluInp.PREV_DELAY_4, AluInp.PREV_DELAY_1)
        b[2].enable_delay_from_src(DelayInp.PREV_ALU_OUT, 2)
        b[2].pass_through_delay(0, 3)
        b[3].pass_through_delay(0, 2, 3)
        for i in range(4, 8):
            b[i].pass_through_alu()
            b[i].pass_through_delay(0, 2, 3)
        return u

    seed = common(UopConfig())
    seed.datapath_config[3].enable_alu(
        AluOp.BYPASS, AluInp.PREV_DELAY_3, AluInp.PREV_DELAY_3
    )
    seed.repeat_count = 1
    seed.trigger = (Trigger.COUNT, Trigger.NONE, Trigger.NONE)
    seed.next_uop = (1, 0, 0)

    steady = common(UopConfig())
    steady.datapath_config[3].enable_alu(
        AluOp.MIN, AluInp.CURR_ALU_OUT, AluInp.PREV_ALU_OUT
    )
    steady.trigger = (Trigger.SRC_TENSOR_DONE, Trigger.NONE, Trigger.NONE)
    steady.next_uop = (2, 0, 0)
    steady.require_inp0 = 1
    steady.require_inp1 = 1
    steady.enable_output(OutSel.DELAY_0, OutPath.WR0_LO)
    steady.enable_output(OutSel.DELAY_2, OutPath.WR0_HI)

    def drain(u: UopConfig):
        b = u.datapath_config
        b[3].enable_alu(AluOp.BYPASS, AluInp.CURR_ALU_OUT, AluInp.CURR_ALU_OUT)
        for i in range(4, 8):
            b[i].pass_through_alu()
        u.trigger = (Trigger.COUNT, Trigger.NONE, Trigger.NONE)
        return u

    drain_a = drain(UopConfig())
    drain_a.repeat_count = DRAIN_A
    drain_a.next_uop = (3, 0, 0)

    drain_b = drain(UopConfig())
    drain_b.repeat_count = 1
    drain_b.next_uop = (0, 0, 0)
    drain_b.enable_output(OutSel.ALU_OUT, OutPath.WR0_LO)
    drain_b.enable_output(OutSel.ALU_OUT, OutPath.WR0_HI)

    return [seed, steady, drain_a, drain_b]


class _FusedOp:
    name = OP_NAME
    spec = _SPEC
    subdim = False

    def __init__(self):
        self._cache = {}

    def compile(self, ver):
        if ver in self._cache:
            return self._cache[ver]
        assert ver == "v3", f"only TRN2/v3 supported, got {ver}"
        s = DveOpSpec(
            name=self.name,
            opcode=DO.get_dve_sub_opcode(self.name),
            uops=_build_1x(),
            uops_2x=_build_2x(),
            rd1_en=True,
            perf_max=1,
        )
        s.validate(ver)
        self._cache[ver] = s
        return s


def _register():
    if OP_NAME in DO._SUB_OPCODE_FOR_NAME:
        return next(op for op in DO.OPS if op.name == OP_NAME)
    op = _FusedOp()
    DO.OPS.append(op)
    DO._SUB_OPCODE_FOR_NAME[OP_NAME] = DO._CUSTOM_DVE_ROW_BASE + len(DO.OPS) - 1
    DO.CUSTOM_DVE_SPECS[OP_NAME] = _SPEC
    return op


FUSED_OP = _register()


def emit_fused(nc, out, in0, in1, s0):
    inst = nc.vector._custom_dve(FUSED_OP, out=out, in0=in0, in1=in1, s0=s0, s1=0.0)
    inst.ins.perf_max = 1  # BassInstruction wraps the rust instr as .ins
    return inst


# --------------------------------------------------------------------------
# Chamfer kernel
# --------------------------------------------------------------------------

B, N, M, D = 4, 8192, 8192, 64
N_CORES = 8
ROWS = N // 2          # rows per core (half a batch)
SHIFT = 48.0
BIGVAL = 60000.0       # row-min accumulator seed (fp16-safe "+inf")

P = 128                # n-tile height (PSUM partitions)
MB = 512               # m-block width (one PSUM bank of fp32)
GROUP = 4              # m-blocks per PSUM group tile ([128, 2048] = 4 banks)
PAD = 2                # row-min pad columns appended to each C group

# every k-th eligible tile (i>0) goes PSUM-direct on the DVE (no ACT copy)
K_PSUM = int(os.environ.get("CHAMFER_K_PSUM", "27"))

LAST_RESULTS = None    # test.py reads exec_time_ns / profile from here


def _build_program(rows=ROWS, cols=M):
    n_tiles = rows // P
    m_groups = cols // (MB * GROUP)
    GW = MB * GROUP        # feed-group width (2048)
    K = D + 2

    f16 = mybir.dt.float16
    f32 = mybir.dt.float32

    # choose PSUM-direct tiles: spread K_PSUM of the i>0 tiles evenly
    # (i-major linear index: lin = i * m_groups + g; skip i == 0)
    n_lin = n_tiles * m_groups
    eligible = [t for t in range(m_groups, n_lin)]
    psum_path = set()
    if K_PSUM > 0:
        stride = len(eligible) / K_PSUM
        psum_path = {eligible[min(len(eligible) - 1, int(j * stride))]
                     for j in range(K_PSUM)}

    nc = bacc.Bacc()
    lhs_d = nc.dram_tensor("lhs", [K, rows], f16, kind="ExternalInput")
    rhs_d = nc.dram_tensor("rhs", [K, cols], f16, kind="ExternalInput")
    row_d = nc.dram_tensor("rowacc", [n_tiles, m_groups, P, PAD], f16,
                           kind="ExternalOutput")
    col_d = nc.dram_tensor("colmins", [P, cols], f16, kind="ExternalOutput")

    with tile.TileContext(nc) as tc:
        with (
            tc.tile_pool(name="const", bufs=1) as const_pool,
            tc.tile_pool(name="feed", bufs=8) as feed_pool,
            tc.tile_pool(name="psum", bufs=2, space="PSUM") as psum_pool,
        ):
            lhs_sb = const_pool.tile([K, rows], f16)
            rhs_sb = const_pool.tile([K, cols], f16)
            # chunked loads, ordered so the first matmul (lhs cols 0:128 +
            # rhs cols 0:512) gates on the first two DMAs, not the whole train
            nc.sync.dma_start(lhs_sb[:, 0:P], lhs_d[:, 0:P])
            for c in range(0, GW, MB):
                nc.sync.dma_start(rhs_sb[:, c:c + MB], rhs_d[:, c:c + MB])
            nc.sync.dma_start(lhs_sb[:, P:GW], lhs_d[:, P:GW])
            lhs_chunks = [(c, min(c + GW, rows)) for c in range(GW, rows, GW)]
            rhs_chunks = [(c, min(c + GW, cols)) for c in range(GW, cols, GW)]
            li = ri = 0
            while ri < len(rhs_chunks) or li < len(lhs_chunks):
                if ri < len(rhs_chunks):
                    c, e = rhs_chunks[ri]; ri += 1
                    nc.sync.dma_start(rhs_sb[:, c:e], rhs_d[:, c:e])
                if li < len(lhs_chunks):
                    c, e = lhs_chunks[li]; li += 1
                    nc.sync.dma_start(lhs_sb[:, c:e], lhs_d[:, c:e])

            # col-min accumulators: ping-pong pair per m-group so the pad
            # eviction DMA of tile i never WAR-blocks tile i+1's op
            Cs = [
                [
                    const_pool.tile([P, GW + PAD], f16, name=f"C{g}_{s}")
                    for s in range(2)
                ]
                for g in range(m_groups)
            ]

            for i in range(n_tiles):
                lhs_i = lhs_sb[:, ts(i, P)]
                for g in range(m_groups):
                    lin = i * m_groups + g
                    Cg_out = Cs[g][i % 2]
                    Cg_in = Cs[g][(i + 1) % 2]
                    ps = psum_pool.tile([P, GW], f32)
                    for jj in range(GROUP):
                        j = g * GROUP + jj
                        nc.tensor.matmul(
                            ps[:, ts(jj, MB)],
                            lhs_i,
                            rhs_sb[:, ts(j, MB)],
                            start=True,
                            stop=True,
                        )
                    if lin in psum_path:
                        # DVE reads PSUM directly (1x program): drain + both
                        # min passes in one op, ScalarE untouched
                        emit_fused(nc, out=Cg_out[:], in0=ps[:],
                                   in1=Cg_in[:, 0:GW], s0=BIGVAL)
                    else:
                        sb = feed_pool.tile([P, GW], f16)
                        nc.scalar.copy(sb[:], ps[:])
                        if i == 0:
                            # C is uninitialized: min(feed, feed) = feed
                            emit_fused(nc, out=Cg_out[:], in0=sb[:],
                                       in1=sb[:], s0=BIGVAL)
                        else:
                            emit_fused(nc, out=Cg_out[:], in0=sb[:],
                                       in1=Cg_in[:, 0:GW], s0=BIGVAL)
                    # evict this tile's row-min pad (ping-pong: next op on
                    # this group writes the other buffer, so no WAR stall)
                    nc.sync.dma_start(row_d[i, g], Cg_out[:, GW:GW + PAD])

            # final col-min stores: fan the 2MB across 4 engine DMA queues so
            # the tail drains in parallel instead of serially on one queue
            last = (n_tiles - 1) % 2
            engines = [nc.sync, nc.gpsimd, nc.scalar, nc.sync]
            for g in range(m_groups):
                engines[g % len(engines)].dma_start(
                    col_d[:, ts(g, GW)], Cs[g][last][:, 0:GW]
                )

    nc.finalize()
    return nc


_PROGRAM_CACHE = {}


def _get_program(rows=ROWS, cols=M):
    key = (rows, cols, K_PSUM)
    if key not in _PROGRAM_CACHE:
        _PROGRAM_CACHE[key] = _build_program(rows, cols)
    return _PROGRAM_CACHE[key]


def _prep_core_inputs(f, f_, core):
    """Host-side shard + layout: build augmented lhs/rhs for one core."""
    b, h = divmod(core, 2)
    fh = f[b, h * ROWS : (h + 1) * ROWS]          # [ROWS, D]
    g = f_[b]                                     # [M, D]
    p = np.einsum("nd,nd->n", fh, fh, dtype=np.float32)
    q = np.einsum("md,md->m", g, g, dtype=np.float32)

    K = D + 2
    lhs = np.empty((K, ROWS), np.float16)
    lhs[:D] = (-2.0 * fh.T).astype(np.float16)
    lhs[D] = p.astype(np.float16)
    lhs[D + 1] = 1.0

    rhs = np.empty((K, M), np.float16)
    rhs[:D] = g.T.astype(np.float16)
    rhs[D] = 1.0
    rhs[D + 1] = (q - SHIFT).astype(np.float16)
    return {"lhs": lhs, "rhs": rhs}


def kernel(f, f_):
    global LAST_RESULTS
    f = np.asarray(f, dtype=np.float32)
    f_ = np.asarray(f_, dtype=np.float32)

    in_maps = [_prep_core_inputs(f, f_, c) for c in range(N_CORES)]
    nc = _get_program()
    res = run_bass_kernel_spmd(
        nc,
        in_maps,
        list(range(N_CORES)),
        trace=bool(int(os.environ.get("CHAMFER_TRACE", "0"))),
    )
    LAST_RESULTS = res

    total = 0.0
    for b in range(B):
        r0 = res.results[2 * b]
        r1 = res.results[2 * b + 1]
        # rowacc[i, g, p, 0] = row-min over group g for row i*128 + p
        rm = np.concatenate(
            [
                r0["rowacc"][:, :, :, 0].astype(np.float32).min(axis=1).reshape(-1),
                r1["rowacc"][:, :, :, 0].astype(np.float32).min(axis=1).reshape(-1),
            ]
        ) + SHIFT
        cm = (
            np.minimum(
                r0["colmins"].astype(np.float32).min(axis=0),
                r1["colmins"].astype(np.float32).min(axis=0),
            )
            + SHIFT
        )
        total += rm.mean() + cm.mean()
    return np.asarray(total / B, dtype=np.float32)
